# revision 41
# baseline (speedup 1.0000x reference)
"""Trainium2 Bass kernel for nn_Attention_5935644803277 (CvT-style sparse attention).

Full-input contract: kernel(**inputs) takes the unsharded inputs (x: [32,1536,768])
and returns the full output [32,1536,768]. Internally shards batch 32 -> 4 per core
across 8 NeuronCores (SPMD, no collectives).

Math (per batch):
  tpl = x[:256] as 16x16 image, onl = x[256:512] as 16x16, srch = x[512:] as 32x32
  q = concat(dwconv3x3_s1(img) for img) -> BN -> @ wq.T   (1536 tokens)
  k,v = same with stride 2 -> 384 tokens
  heads(12, hd=64); templates (first 512 q) attend to first 128 k/v;
  search (last 1024 q) attend to all 384; softmax(QK^T * 768^-0.5);
  out = concat @ w_proj.T + b_proj

Performance architecture (v2):
  - Scores here are tiny (|s|*scale ~ 0.05) so the whole Q/K path is
    numerically immune to fp8 quantization; the V/output path is not.
  - fp8e4 DoubleRow matmuls (0.5 cyc/out-row) for: conv-q + conv-k
    (diagonal depthwise matmuls, conv-weight hi/lo residual pair in the two
    k-tiles), q/k projections (genuine 2x128-contraction pairs), attention
    scores (zero second k-tile), A@V (v hi + lo residual k-tiles -> v is
    exact to f32-PSUM precision), and the output bias (bias hi/lo pair).
  - bf16 for the error-carrying path: conv-v, V projection, xatt, out proj.
  - Softmax: exp(s*scale - 2) on ACT straight to fp8 (the shift cancels in
    the normalize divide); denominator via 64 ones-columns in the V
    stationary; normalize = one DVE divide (num/den) instead of recip+mul.
  - conv diagonals are persistent SBUF constants (no per-batch reload).
"""
import numpy as np

import concourse.bass as bass
import concourse.tile as tile
from concourse import bacc, mybir
from concourse.bass_utils import run_bass_kernel_spmd

F32 = mybir.dt.float32
BF16 = mybir.dt.bfloat16
F8 = mybir.dt.float8e4
AF = mybir.ActivationFunctionType
OP = mybir.AluOpType
PM = mybir.MatmulPerfMode

EPS = 1e-5
NB = 4          # batches per core
L = 1536
D = 768
G = 6           # channel chunks of 128
NH = 12
HD = 64
SCALE = float(D) ** -0.5
LKV = 384
ESHIFT = 2.0    # exp(s*SCALE - ESHIFT); cancels in the softmax divide

TAPS = [(1, 1)] + [(dh, dw) for dh in range(3) for dw in range(3)
                   if (dh, dw) != (1, 1)]  # center first: covers full PSUM region


def _rect(tile_ap, base, dims):
    """AP keeping the partition dim of tile_ap, with new free dims at
    free-offset base (in elements)."""
    return bass.AP(tensor=tile_ap.tensor, offset=tile_ap.offset + base,
                   ap=[list(tile_ap.ap[0])] + [list(d) for d in dims])


def _slot(sliced, zoff, n=None):
    """Insert a DoubleRow k-tile dim [zoff, 2] before the last free dim of a
    [partition, last] AP."""
    ap = [list(d) for d in sliced.ap]
    assert len(ap) == 2
    last = ap[1] if n is None else [ap[1][0], n]
    return bass.AP(tensor=sliced.tensor, offset=sliced.offset,
                   ap=[ap[0], [zoff, 2], last])


def _tap_bounds(d, H):
    """stride-1 output row range for tap offset d in {0,1,2}."""
    r0 = 1 if d == 0 else 0
    r1 = H - 2 if d == 2 else H - 1
    return r0, r1 - r0 + 1


def _tap_bounds_s2(d, H):
    """stride-2: output rows where input row 2r+d-1 in [0,H). H even."""
    Ho = H // 2
    r0 = 1 if d == 0 else 0
    return r0, Ho - r0


def build_program(nb=NB, reps=1):
    nc = bacc.Bacc("TRN2", target_bir_lowering=False, debug=False, num_devices=8)

    x_d = nc.dram_tensor("x", [nb, L, D], F32, kind="ExternalInput").ap()
    wq8_d = nc.dram_tensor("wq8", [128, G, D], F8, kind="ExternalInput").ap()
    wk8_d = nc.dram_tensor("wk8", [128, G, D], F8, kind="ExternalInput").ap()
    wv_d = nc.dram_tensor("wv", [128, G, D], BF16, kind="ExternalInput").ap()
    wp_d = nc.dram_tensor("wp", [128, G, D], BF16, kind="ExternalInput").ap()
    bq_d = nc.dram_tensor("bq", [128, G], F32, kind="ExternalInput").ap()
    bfin8_d = nc.dram_tensor("bfin8", [1, 2, D], F8, kind="ExternalInput").ap()
    bfinb_d = nc.dram_tensor("bfinb", [1, D], BF16, kind="ExternalInput").ap()
    ones18_d = nc.dram_tensor("ones18", [1, 2, 128], F8, kind="ExternalInput").ap()
    dgq_d = nc.dram_tensor("dgq", [128, G, 10, 128], F8, kind="ExternalInput").ap()
    dgk_d = nc.dram_tensor("dgk", [128, G, 10, 128], F8, kind="ExternalInput").ap()
    dgv_d = nc.dram_tensor("dgv", [128, G, 9, 128], BF16, kind="ExternalInput").ap()
    out_d = nc.dram_tensor("out", [nb, L, D], F32, kind="ExternalOutput").ap()

    with tile.TileContext(nc) as tc:
        with (
            tc.tile_pool(name="consts", bufs=1) as consts,
            tc.tile_pool(name="act", bufs=1) as actp,
            tc.tile_pool(name="roll", bufs=1) as roll,
            tc.tile_pool(name="ps_big", bufs=2, space="PSUM") as ps_big,   # sps [128,1024]
            tc.tile_pool(name="ps_pj", bufs=2, space="PSUM") as ps_pj,     # conv/proj/opj
            tc.tile_pool(name="ps_u", bufs=2, space="PSUM") as ps_u,       # A@V num+den
            tc.tile_pool(name="dram", bufs=2, space="DRAM") as dramp,
        ):
            # ---------------- constants ----------------
            # consts DMAs are issued from the (startup-idle) ACT and DVE
            # queues so the SP queue is free for batch-0's transposes.
            bq_sb = consts.tile([128, G], F32, name="bq_sb")
            nc.sync.dma_start(out=bq_sb, in_=bq_d)
            bfin8_sb = consts.tile([1, 2, D], F8, name="bfin8_sb")
            nc.sync.dma_start(out=bfin8_sb, in_=bfin8_d)
            ones18_sb = consts.tile([1, 2, 128], F8, name="ones18_sb")
            nc.sync.dma_start(out=ones18_sb, in_=ones18_d)
            shift_sb = consts.tile([128, 1], F32, name="shift_sb")
            nc.vector.memset(shift_sb, -ESHIFT)
            bfinb_sb = consts.tile([1, D], BF16, name="bfinb_sb")
            nc.sync.dma_start(out=bfinb_sb, in_=bfinb_d)
            ones1b_sb = consts.tile([1, 128], BF16, name="ones1b_sb")
            nc.vector.memset(ones1b_sb, 1.0)
            dgq_sb = consts.tile([128, G, 10, 128], F8, name="dgq_sb")
            dgk_sb = consts.tile([128, G, 10, 128], F8, name="dgk_sb")
            dgv_sb = consts.tile([128, G, 9, 128], BF16, name="dgv_sb")
            wq8_sb = consts.tile([128, G, D], F8, name="wq8_sb")
            wk8_sb = consts.tile([128, G, D], F8, name="wk8_sb")
            wv_sb = consts.tile([128, G, D], BF16, name="wv_sb")
            wp_sb = consts.tile([128, G, D], BF16, name="wp_sb")

            def emit_const_dmas():
                # dgq gates the first conv matmuls: use the startup-idle SP
                # (HWDGE) queue for it; the rest go via SWDGE.
                for g in range(G):
                    eng = nc.sync if g % 2 == 0 else nc.scalar
                    eng.dma_start(out=dgq_sb[:, g], in_=dgq_d[:, g])
                for g in range(G):
                    nc.gpsimd.dma_start(out=dgk_sb[:, g], in_=dgk_d[:, g])
                    nc.gpsimd.dma_start(out=dgv_sb[:, g], in_=dgv_d[:, g])
                nc.gpsimd.dma_start(out=wq8_sb, in_=wq8_d)
                nc.gpsimd.dma_start(out=wk8_sb, in_=wk8_d)
                nc.gpsimd.dma_start(out=wv_sb, in_=wv_d)
                nc.gpsimd.dma_start(out=wp_sb, in_=wp_d)

            # ------------- conv emitters -------------
            def conv_q_mms(psum, xt8s, g, ct):
                gb = g * L
                for i, (dh, dw) in enumerate(TAPS):
                    tap = 3 * dh + dw
                    dg = _slot(dgq_sb[:, g, tap, :], (9 - tap) * 128)
                    if ct == 0:
                        r0, nr = _tap_bounds(dh, 16)
                        c0, nw = _tap_bounds(dw, 16)
                        for k, ib in enumerate((0, 256)):
                            ia = _rect(xt8s, gb + ib + (r0 + dh - 1) * 16 + (c0 + dw - 1),
                                       [[0, 2], [16, nr], [1, nw]])
                            oa = _rect(psum, ib + r0 * 16 + c0, [[16, nr], [1, nw]])
                            nc.tensor.matmul(oa, dg, ia, start=(i == 0 and k == 0),
                                             stop=(i == 8 and k == 1),
                                             perf_mode=PM.DoubleRow,
                                             skip_group_check=True)
                    else:
                        row_lo, row_hi = 16 * (ct - 1), 16 * ct
                        r0, nr = _tap_bounds(dh, 32)
                        rr1 = min(r0 + nr - 1, row_hi - 1)
                        r0 = max(r0, row_lo)
                        nr = rr1 - r0 + 1
                        c0, nw = _tap_bounds(dw, 32)
                        ia = _rect(xt8s, gb + 512 + (r0 + dh - 1) * 32 + (c0 + dw - 1),
                                   [[0, 2], [32, nr], [1, nw]])
                        oa = _rect(psum, (r0 - row_lo) * 32 + c0, [[32, nr], [1, nw]])
                        nc.tensor.matmul(oa, dg, ia, start=(i == 0), stop=(i == 8),
                                         perf_mode=PM.DoubleRow,
                                         skip_group_check=True)

            def conv_kv_mms(psum, xsrc, g, fp8):
                gb = g * L
                n_mm = 27 if fp8 else 18
                mm_i = 0
                for dh, dw in TAPS:
                    t = 3 * dh + dw
                    dg = (_slot(dgk_sb[:, g, t, :], (9 - t) * 128) if fp8
                          else dgv_sb[:, g, t, :])
                    slot = [[0, 2]] if fp8 else []
                    pm = PM.DoubleRow if fp8 else None
                    r0, nr = _tap_bounds_s2(dh, 16)
                    c0, nw = _tap_bounds_s2(dw, 16)
                    if fp8:
                        for ib, ob in ((0, 0), (256, 64)):
                            ia = _rect(xsrc, gb + ib + (2 * r0 + dh - 1) * 16
                                       + (2 * c0 + dw - 1),
                                       slot + [[32, nr], [2, nw]])
                            oa = _rect(psum, ob + r0 * 8 + c0, [[8, nr], [1, nw]])
                            nc.tensor.matmul(oa, dg, ia, start=(mm_i == 0),
                                             stop=False, perf_mode=pm,
                                             skip_group_check=True)
                            mm_i += 1
                    else:
                        ia = _rect(xsrc, gb + (2 * r0 + dh - 1) * 16 + (2 * c0 + dw - 1),
                                   [[256, 2], [32, nr], [2, nw]])
                        oa = _rect(psum, r0 * 8 + c0, [[64, 2], [8, nr], [1, nw]])
                        nc.tensor.matmul(oa, dg, ia, start=(mm_i == 0), stop=False,
                                         perf_mode=pm, skip_group_check=True)
                        mm_i += 1
                    r0, nr = _tap_bounds_s2(dh, 32)
                    c0, nw = _tap_bounds_s2(dw, 32)
                    ia = _rect(xsrc, gb + 512 + (2 * r0 + dh - 1) * 32 + (2 * c0 + dw - 1),
                               slot + [[64, nr], [2, nw]])
                    oa = _rect(psum, 128 + r0 * 16 + c0, [[16, nr], [1, nw]])
                    nc.tensor.matmul(oa, dg, ia, start=False, stop=(mm_i == n_mm - 1),
                                     perf_mode=pm, skip_group_check=True)
                    mm_i += 1

            # ------------- per-batch stage emitters -------------
            state = {}

            def emit_dma_stage(b, rb):
                st = {}
                st["xt"] = actp.tile([128, G, L], BF16, name=f"xt_{rb}", tag="xt",
                                     bufs=2)
                st["xt8"] = actp.tile([128, G, L], F8, name=f"xt8_{rb}", tag="xt8",
                                      bufs=2)
                xbf = dramp.tile([L, D], BF16, name=f"xbf_{rb}", tag="xbf", bufs=2)
                for g in range(G):
                    nc.gpsimd.dma_start(out=xbf[:, g * 128:(g + 1) * 128],
                                        in_=x_d[b][:, g * 128:(g + 1) * 128])
                    nc.sync.dma_start_transpose(
                        st["xt"][:, g, :], xbf[:, g * 128:(g + 1) * 128])
                    if rb == "0":
                        # startup: Pool queue is busy with consts; DVE and ACT
                        # are idle -- split the casts across both
                        if g % 2 == 0:
                            nc.vector.tensor_copy(out=st["xt8"][:, g],
                                                  in_=st["xt"][:, g])
                        else:
                            nc.scalar.copy(st["xt8"][:, g], st["xt"][:, g])
                    else:
                        nc.gpsimd.tensor_copy(out=st["xt8"][:, g], in_=st["xt"][:, g])
                st["cq8"] = actp.tile([128, G, L], F8, name=f"cq8_{rb}", tag="cq8",
                                      bufs=2)
                st["ck8"] = actp.tile([128, G, LKV], F8, name=f"ck8_{rb}", tag="ck8",
                                      bufs=2)
                st["cv"] = actp.tile([128, G, LKV], BF16, name=f"cv_{rb}", tag="cv",
                                     bufs=2)
                st["kt8"] = actp.tile([128, G + 1, LKV], F8, name=f"kt8_{rb}",
                                      tag="kt8", bufs=2)
                st["v8"] = actp.tile([128, 3, NH, 2, 128], F8, name=f"v8_{rb}",
                                     tag="v8", bufs=2)
                if int(rb) < 2:
                    # ones/zero blocks live in disjoint columns from the evacs,
                    # so each pool buffer only needs them written once
                    nc.gpsimd.memset(st["kt8"][:, G, :], 0.0)
                    nc.gpsimd.memset(_rect(st["v8"], HD,
                                           [[3072, 3], [256, NH], [1, HD]]), 1.0)
                    nc.gpsimd.memset(_rect(st["v8"], 128 + HD,
                                           [[3072, 3], [256, NH], [1, HD]]), 0.0)
                state[b] = st

            def h1_chunks(b, rb):
                """PE-dense conv + k/v projection work, as filler closures."""
                st = state[b]
                chunks = []
                for g in range(G):
                    def convq_g(g=g):
                        for ct in range(3):
                            pj = ps_pj.tile([128, 512], F32,
                                            name=f"cqp_{rb}_{g}_{ct}", tag="pj")
                            conv_q_mms(pj, st["xt8"], g, ct)
                            nc.vector.tensor_copy(
                                out=st["cq8"][:, g, ct * 512:(ct + 1) * 512], in_=pj)
                    chunks.append(convq_g)
                for g in range(G):
                    def convkv_g(g=g):
                        pk = ps_pj.tile([128, LKV], F32, name=f"ckp_{rb}_{g}",
                                        tag="pj")
                        conv_kv_mms(pk, st["xt8"], g, fp8=True)
                        nc.scalar.copy(st["ck8"][:, g, :], pk)
                        pv = ps_pj.tile([128, LKV], F32, name=f"cvp_{rb}_{g}",
                                        tag="pj")
                        conv_kv_mms(pv, st["xt"], g, fp8=False)
                        nc.scalar.copy(st["cv"][:, g, :], pv)
                    chunks.append(convkv_g)
                for go in range(G):
                    def kproj_go(go=go):
                        pj = ps_pj.tile([128, LKV], F32, name=f"kpj_{rb}_{go}",
                                        tag="pj")
                        for m in range(3):
                            lw = _slot(wk8_sb[:, 2 * m, go * 128:(go + 1) * 128], D)
                            ra = _slot(st["ck8"][:, 2 * m, :], LKV)
                            nc.tensor.matmul(pj, lw, ra, start=(m == 0), stop=(m == 2),
                                             perf_mode=PM.DoubleRow)
                        nc.vector.tensor_copy(out=st["kt8"][:, go, :], in_=pj)
                    chunks.append(kproj_go)
                for mt in range(3):
                    def vproj_mt(mt=mt):
                        for nh in range(2):
                            pj = ps_pj.tile([128, LKV], F32,
                                            name=f"vpj_{rb}_{mt}_{nh}", tag="pj")
                            for g in range(G):
                                nc.tensor.matmul(
                                    pj, st["cv"][:, g, mt * 128:(mt + 1) * 128],
                                    wv_sb[:, g, nh * 384:(nh + 1) * 384],
                                    start=(g == 0), stop=(g == G - 1))
                            hi = _rect(st["v8"], mt * 3072 + nh * 6 * 256,
                                       [[256, 6], [1, HD]])
                            nc.scalar.copy(hi, pj)
                            lo = _rect(st["v8"], mt * 3072 + nh * 6 * 256 + 128,
                                       [[256, 6], [1, HD]])
                            nc.vector.tensor_tensor(out=lo, in0=pj, in1=hi,
                                                    op=OP.subtract)
                    chunks.append(vproj_mt)
                return chunks

            def emit_h2(b, rb, filler, pending):
                """Latency-bound attention; drains filler chunks (next batch's
                conv work) into the PE stream to cover softmax latency."""
                st = state[b]
                cq8, kt8, v8 = st["cq8"], st["kt8"], st["v8"]

                def drain(n=1):
                    for _ in range(n):
                        c = next(filler, None)
                        if c is not None:
                            c()

                def emit_outproj(tp, xatt_t, mt2):
                    onat = roll.tile([128, D], F32, name=f"on_{rb}_{tp}_{mt2}",
                                     tag="onat", bufs=3)
                    for nh in range(2):
                        pj = ps_pj.tile([128, 384], F32,
                                        name=f"opj_{rb}_{tp}_{mt2}_{nh}", tag="pj")
                        nc.tensor.matmul(pj, ones1b_sb,
                                         bfinb_sb[:, nh * 384:(nh + 1) * 384],
                                         start=True, stop=False)
                        for g in range(G):
                            nc.tensor.matmul(
                                pj, xatt_t[:, g, mt2 * 128:(mt2 + 1) * 128],
                                wp_sb[:, g, nh * 384:(nh + 1) * 384],
                                start=False, stop=(g == G - 1))
                        if nh == 0:
                            nc.scalar.copy(onat[:, 0:384], pj)
                        else:
                            nc.vector.tensor_copy(out=onat[:, 384:768], in_=pj)
                    tok0 = tp * 512 + mt2 * 128
                    nc.sync.dma_start(out=out_d[b, tok0:tok0 + 128, :], in_=onat)

                def qproj_chunk(t, qt8, go):
                    pj = ps_pj.tile([128, 512], F32,
                                    name=f"qpj_{rb}_{t}_{go}", tag="pj")
                    for m in range(3):
                        lw = _slot(wq8_sb[:, 2 * m, go * 128:(go + 1) * 128], D)
                        ra = _slot(cq8[:, 2 * m, t * 512:(t + 1) * 512], L)
                        nc.tensor.matmul(pj, lw, ra, start=(m == 0), stop=(m == 2),
                                         perf_mode=PM.DoubleRow)
                    nc.scalar.activation(qt8[:, go, :], pj, AF.Identity,
                                         bias=bq_sb[:, go:go + 1])

                xatt_tiles = {}
                qt8_next = roll.tile([128, G, 512], F8, name=f"qt8_{rb}_0",
                                     tag="qt8", bufs=2)
                for go in range(G):
                    qproj_chunk(0, qt8_next, go)
                for t in range(3):
                    qt8 = qt8_next
                    if t < 2:
                        qt8_next = roll.tile([128, G, 512], F8,
                                             name=f"qt8_{rb}_{t + 1}",
                                             tag="qt8", bufs=2)
                    drain(1)

                    xatt = roll.tile([128, G, 512], BF16, name=f"xatt_{rb}_{t}",
                                     tag="xatt", bufs=2)
                    xatt_tiles[t] = xatt
                    kcs = (0,) if t == 0 else (0, 1, 2)
                    for hp in range(6):
                        g = hp
                        ups = [ps_u.tile([128, 512], F32,
                                         name=f"u_{rb}_{t}_{hp}_{j}", tag="u")
                               for j in range(2)]

                        def emit_scores(kc):
                            sps = ps_big.tile([128, 1024], F32,
                                              name=f"s_{rb}_{t}_{hp}_{kc}", tag="big")
                            for j, po in enumerate((0, HD)):
                                zoff = (G - g) * LKV - kc * 128
                                lw = _slot(kt8[po:po + HD, g,
                                               kc * 128:(kc + 1) * 128], zoff)
                                ra = _slot(qt8[po:po + HD, g, :], 0)
                                nc.tensor.matmul(
                                    sps[:, j * 512:(j + 1) * 512], lw, ra,
                                    start=True, stop=True, perf_mode=PM.DoubleRow)
                            aT8 = roll.tile([128, 1024], F8,
                                            name=f"aT_{rb}_{t}_{hp}_{kc}",
                                            tag="aT", bufs=6)
                            nc.scalar.activation(aT8, sps, AF.Exp, scale=SCALE,
                                                 bias=shift_sb)
                            return aT8

                        def emit_av(kc, aT8, i):
                            for j in range(2):
                                lw = v8[:, kc, 2 * hp + j]
                                ra = _slot(aT8[:, j * 512:(j + 1) * 512], 0)
                                nc.tensor.matmul(ups[j], lw, ra,
                                                 start=(i == 0),
                                                 stop=(i == len(kcs) - 1),
                                                 perf_mode=PM.DoubleRow)

                        aTs = []
                        for i, kc in enumerate(kcs):
                            aTs.append(emit_scores(kc))
                            if i >= 1:
                                emit_av(kcs[i - 1], aTs[i - 1], i - 1)
                        emit_av(kcs[-1], aTs[-1], len(kcs) - 1)
                        for j in range(2):
                            recip = roll.tile([HD, 512], F32,
                                              name=f"rc_{rb}_{t}_{hp}_{j}",
                                              tag="recip", bufs=3)
                            nc.vector.reciprocal(recip, ups[j][HD:2 * HD, :])
                            nc.vector.tensor_mul(
                                xatt[j * HD:(j + 1) * HD, g, :],
                                ups[j][0:HD, :], recip)
                        drain(1)
                        if t < 2:
                            qproj_chunk(t + 1, qt8_next, hp)
                        if t > 0 and 1 <= hp <= 4:
                            emit_outproj(t - 1, xatt_tiles[t - 1], hp - 1)
                        elif t == 0 and 1 <= hp <= 4 and pending:
                            pending.pop(0)()
                while pending:
                    pending.pop(0)()
                # defer this batch's last-tile outproj into the next batch's
                # t=0 attention window (PE filler there); emit inline if last.
                xt2 = xatt_tiles[2]
                new_pending = [
                    (lambda mt2=mt2: emit_outproj(2, xt2, mt2)) for mt2 in range(4)]
                return new_pending

            # ------------- pipelined schedule -------------
            bseq = [bb for _ in range(reps) for bb in range(nb)]
            n = len(bseq)
            emit_dma_stage(bseq[0], "0")
            emit_const_dmas()
            for c in h1_chunks(bseq[0], "0"):
                c()
            pending = []
            for i in range(n):
                if i + 1 < n:
                    emit_dma_stage(bseq[i + 1], f"{i + 1}")
                    filler = iter(h1_chunks(bseq[i + 1], f"{i + 1}"))
                else:
                    filler = iter(())
                pending = emit_h2(bseq[i], f"{i}", filler, pending)
                for c in filler:
                    c()
                state.pop(bseq[i], None)
            for c in pending:
                c()

    nc.compile()
    return nc


_NC_CACHE = {}


def _get_program():
    if "nc" not in _NC_CACHE:
        _NC_CACHE["nc"] = build_program()
    return _NC_CACHE["nc"]


def _host_prep(inputs):
    import ml_dtypes
    MLF8 = ml_dtypes.float8_e4m3
    MLBF = ml_dtypes.bfloat16

    f = lambda k: np.asarray(inputs[k], dtype=np.float32)
    w = {}
    effs = {}
    for n in ("q", "k", "v"):
        inv = f(f"bn_{n}_g") / np.sqrt(f(f"bn_{n}_v") + EPS)
        beta = f(f"bn_{n}_b") - f(f"bn_{n}_m") * inv
        effs[n] = (inv, beta)
    wq, wk, wv, wp = f("wq"), f("wk"), f("wv"), f("w_proj")
    wqT = np.ascontiguousarray((wq * effs["q"][0][None, :]).T)  # [in, out]
    wkT = np.ascontiguousarray((wk * effs["k"][0][None, :]).T)
    wvT = np.ascontiguousarray((wv * effs["v"][0][None, :]).T)
    wpT = np.ascontiguousarray(wp.T)
    w["wq8"] = wqT.reshape(G, 128, D).transpose(1, 0, 2).astype(MLF8)
    w["wk8"] = wkT.reshape(G, 128, D).transpose(1, 0, 2).astype(MLF8)
    w["wv"] = wvT.reshape(G, 128, D).transpose(1, 0, 2).astype(MLBF)
    w["wp"] = wpT.reshape(G, 128, D).transpose(1, 0, 2).astype(MLBF)
    bq_eff = wq @ effs["q"][1]
    w["bq"] = np.ascontiguousarray(bq_eff.reshape(G, 128).T.astype(np.float32))
    bv_eff = wv @ effs["v"][1]
    b_fin = (f("b_proj") + wp @ bv_eff).astype(np.float32)
    bf_hi = b_fin.astype(MLF8)
    bf_lo = (b_fin - bf_hi.astype(np.float32)).astype(MLF8)
    bfin8 = np.zeros((1, 2, D), MLF8)
    bfin8[0, 0] = bf_hi
    bfin8[0, 1] = bf_lo
    w["bfin8"] = bfin8
    w["bfinb"] = b_fin.reshape(1, D).astype(MLBF)
    w["ones18"] = np.ones((1, 2, 128), np.float32).astype(MLF8)
    eye = np.eye(128, dtype=np.float32)
    for name, key in (("dgq", "conv_q_w"), ("dgk", "conv_k_w")):
        cw = f(key).reshape(D, 9)
        dg = np.zeros((128, G, 10, 128), np.float32)
        for g in range(G):
            dg[:, g, :9, :] = eye[:, None, :] * cw[g * 128:(g + 1) * 128][:, :, None]
        w[name] = dg.astype(MLF8)
    cwv = f("conv_v_w").reshape(D, 9)
    dgv = np.zeros((128, G, 9, 128), np.float32)
    for g in range(G):
        dgv[:, g, :, :] = eye[:, None, :] * cwv[g * 128:(g + 1) * 128][:, :, None]
    w["dgv"] = dgv.astype(MLBF)
    return {k: np.ascontiguousarray(v) for k, v in w.items()}


def kernel(**inputs):
    x = np.asarray(inputs["x"], dtype=np.float32)
    assert x.shape == (32, L, D), x.shape
    const = _host_prep(inputs)
    nc = _get_program()
    in_maps = []
    for c in range(8):
        m = dict(const)
        m["x"] = np.ascontiguousarray(x[c * NB:(c + 1) * NB])
        in_maps.append(m)
    res = run_bass_kernel_spmd(nc, in_maps, list(range(8)))
    out = np.concatenate([res.results[c]["out"] for c in range(8)], axis=0)
    return out.astype(np.float32)


# revision 42
# speedup vs baseline: 1.0171x; 1.0171x over previous
"""Trainium2 Bass kernel for nn_Attention_5935644803277 (CvT-style sparse attention).

Full-input contract: kernel(**inputs) takes the unsharded inputs (x: [32,1536,768])
and returns the full output [32,1536,768]. Internally shards batch 32 -> 4 per core
across 8 NeuronCores (SPMD, no collectives).

Math (per batch):
  tpl = x[:256] as 16x16 image, onl = x[256:512] as 16x16, srch = x[512:] as 32x32
  q = concat(dwconv3x3_s1(img) for img) -> BN -> @ wq.T   (1536 tokens)
  k,v = same with stride 2 -> 384 tokens
  heads(12, hd=64); templates (first 512 q) attend to first 128 k/v;
  search (last 1024 q) attend to all 384; softmax(QK^T * 768^-0.5);
  out = concat @ w_proj.T + b_proj

Performance architecture (v2):
  - Scores here are tiny (|s|*scale ~ 0.05) so the whole Q/K path is
    numerically immune to fp8 quantization; the V/output path is not.
  - fp8e4 DoubleRow matmuls (0.5 cyc/out-row) for: conv-q + conv-k
    (diagonal depthwise matmuls, conv-weight hi/lo residual pair in the two
    k-tiles), q/k projections (genuine 2x128-contraction pairs), attention
    scores (zero second k-tile), A@V (v hi + lo residual k-tiles -> v is
    exact to f32-PSUM precision), and the output bias (bias hi/lo pair).
  - bf16 for the error-carrying path: conv-v, V projection, xatt, out proj.
  - Softmax: exp(s*scale - 2) on ACT straight to fp8 (the shift cancels in
    the normalize divide); denominator via 64 ones-columns in the V
    stationary; normalize = one DVE divide (num/den) instead of recip+mul.
  - conv diagonals are persistent SBUF constants (no per-batch reload).
"""
import numpy as np

import concourse.bass as bass
import concourse.tile as tile
from concourse import bacc, mybir
from concourse.bass_utils import run_bass_kernel_spmd

F32 = mybir.dt.float32
BF16 = mybir.dt.bfloat16
F8 = mybir.dt.float8e4
AF = mybir.ActivationFunctionType
OP = mybir.AluOpType
PM = mybir.MatmulPerfMode

EPS = 1e-5
NB = 4          # batches per core
L = 1536
D = 768
G = 6           # channel chunks of 128
NH = 12
HD = 64
SCALE = float(D) ** -0.5
LKV = 384
ESHIFT = 2.0    # exp(s*SCALE - ESHIFT); cancels in the softmax divide

TAPS = [(1, 1)] + [(dh, dw) for dh in range(3) for dw in range(3)
                   if (dh, dw) != (1, 1)]  # center first: covers full PSUM region


def _rect(tile_ap, base, dims):
    """AP keeping the partition dim of tile_ap, with new free dims at
    free-offset base (in elements)."""
    return bass.AP(tensor=tile_ap.tensor, offset=tile_ap.offset + base,
                   ap=[list(tile_ap.ap[0])] + [list(d) for d in dims])


def _slot(sliced, zoff, n=None):
    """Insert a DoubleRow k-tile dim [zoff, 2] before the last free dim of a
    [partition, last] AP."""
    ap = [list(d) for d in sliced.ap]
    assert len(ap) == 2
    last = ap[1] if n is None else [ap[1][0], n]
    return bass.AP(tensor=sliced.tensor, offset=sliced.offset,
                   ap=[ap[0], [zoff, 2], last])


def _tap_bounds(d, H):
    """stride-1 output row range for tap offset d in {0,1,2}."""
    r0 = 1 if d == 0 else 0
    r1 = H - 2 if d == 2 else H - 1
    return r0, r1 - r0 + 1


def _tap_bounds_s2(d, H):
    """stride-2: output rows where input row 2r+d-1 in [0,H). H even."""
    Ho = H // 2
    r0 = 1 if d == 0 else 0
    return r0, Ho - r0


def build_program(nb=NB, reps=1):
    nc = bacc.Bacc("TRN2", target_bir_lowering=False, debug=False, num_devices=8)

    x_d = nc.dram_tensor("x", [nb, L, D], F32, kind="ExternalInput").ap()
    wq8_d = nc.dram_tensor("wq8", [128, G, D], F8, kind="ExternalInput").ap()
    wk8_d = nc.dram_tensor("wk8", [128, G, D], F8, kind="ExternalInput").ap()
    wv_d = nc.dram_tensor("wv", [128, G, D], BF16, kind="ExternalInput").ap()
    wp_d = nc.dram_tensor("wp", [128, G, D], BF16, kind="ExternalInput").ap()
    bq_d = nc.dram_tensor("bq", [128, G], F32, kind="ExternalInput").ap()
    bfin8_d = nc.dram_tensor("bfin8", [1, 2, D], F8, kind="ExternalInput").ap()
    bfinb_d = nc.dram_tensor("bfinb", [1, D], BF16, kind="ExternalInput").ap()
    ones18_d = nc.dram_tensor("ones18", [1, 2, 128], F8, kind="ExternalInput").ap()
    dgq_d = nc.dram_tensor("dgq", [128, G, 10, 128], F8, kind="ExternalInput").ap()
    dgk_d = nc.dram_tensor("dgk", [128, G, 10, 128], F8, kind="ExternalInput").ap()
    dgv_d = nc.dram_tensor("dgv", [128, G, 9, 128], BF16, kind="ExternalInput").ap()
    out_d = nc.dram_tensor("out", [nb, L, D], F32, kind="ExternalOutput").ap()

    with tile.TileContext(nc) as tc:
        with (
            tc.tile_pool(name="consts", bufs=1) as consts,
            tc.tile_pool(name="act", bufs=1) as actp,
            tc.tile_pool(name="roll", bufs=1) as roll,
            tc.tile_pool(name="ps_big", bufs=2, space="PSUM") as ps_big,   # sps [128,1024]
            tc.tile_pool(name="ps_pj", bufs=2, space="PSUM") as ps_pj,     # conv/proj/opj
            tc.tile_pool(name="ps_u", bufs=2, space="PSUM") as ps_u,       # A@V num+den
            tc.tile_pool(name="dram", bufs=2, space="DRAM") as dramp,
        ):
            # ---------------- constants ----------------
            # consts DMAs are issued from the (startup-idle) ACT and DVE
            # queues so the SP queue is free for batch-0's transposes.
            bq_sb = consts.tile([128, G], F32, name="bq_sb")
            nc.sync.dma_start(out=bq_sb, in_=bq_d)
            bfin8_sb = consts.tile([1, 2, D], F8, name="bfin8_sb")
            nc.sync.dma_start(out=bfin8_sb, in_=bfin8_d)
            ones18_sb = consts.tile([1, 2, 128], F8, name="ones18_sb")
            nc.sync.dma_start(out=ones18_sb, in_=ones18_d)
            shift_sb = consts.tile([128, 1], F32, name="shift_sb")
            nc.vector.memset(shift_sb, -ESHIFT)
            bfinb_sb = consts.tile([1, D], BF16, name="bfinb_sb")
            nc.sync.dma_start(out=bfinb_sb, in_=bfinb_d)
            ones1b_sb = consts.tile([1, 128], BF16, name="ones1b_sb")
            nc.vector.memset(ones1b_sb, 1.0)
            dgq_sb = consts.tile([128, G, 10, 128], F8, name="dgq_sb")
            dgk_sb = consts.tile([128, G, 10, 128], F8, name="dgk_sb")
            dgv_sb = consts.tile([128, G, 9, 128], BF16, name="dgv_sb")
            wq8_sb = consts.tile([128, G, D], F8, name="wq8_sb")
            wk8_sb = consts.tile([128, G, D], F8, name="wk8_sb")
            wv_sb = consts.tile([128, G, D], BF16, name="wv_sb")
            wp_sb = consts.tile([128, G, D], BF16, name="wp_sb")

            def emit_const_dmas():
                # dgq gates the first conv matmuls: use the startup-idle SP
                # (HWDGE) queue for it; the rest go via SWDGE.
                for g in range(G):
                    eng = nc.sync if g % 2 == 0 else nc.scalar
                    eng.dma_start(out=dgq_sb[:, g], in_=dgq_d[:, g])
                for g in range(G):
                    nc.gpsimd.dma_start(out=dgk_sb[:, g], in_=dgk_d[:, g])
                    nc.gpsimd.dma_start(out=dgv_sb[:, g], in_=dgv_d[:, g])
                nc.gpsimd.dma_start(out=wq8_sb, in_=wq8_d)
                nc.gpsimd.dma_start(out=wk8_sb, in_=wk8_d)
                nc.gpsimd.dma_start(out=wv_sb, in_=wv_d)
                nc.gpsimd.dma_start(out=wp_sb, in_=wp_d)

            # ------------- conv emitters -------------
            def conv_q_mms(psum, xt8s, g, ct):
                gb = g * L
                for i, (dh, dw) in enumerate(TAPS):
                    tap = 3 * dh + dw
                    dg = _slot(dgq_sb[:, g, tap, :], (9 - tap) * 128)
                    if ct == 0:
                        r0, nr = _tap_bounds(dh, 16)
                        c0, nw = _tap_bounds(dw, 16)
                        for k, ib in enumerate((0, 256)):
                            ia = _rect(xt8s, gb + ib + (r0 + dh - 1) * 16 + (c0 + dw - 1),
                                       [[0, 2], [16, nr], [1, nw]])
                            oa = _rect(psum, ib + r0 * 16 + c0, [[16, nr], [1, nw]])
                            nc.tensor.matmul(oa, dg, ia, start=(i == 0 and k == 0),
                                             stop=(i == 8 and k == 1),
                                             perf_mode=PM.DoubleRow,
                                             skip_group_check=True)
                    else:
                        row_lo, row_hi = 16 * (ct - 1), 16 * ct
                        r0, nr = _tap_bounds(dh, 32)
                        rr1 = min(r0 + nr - 1, row_hi - 1)
                        r0 = max(r0, row_lo)
                        nr = rr1 - r0 + 1
                        c0, nw = _tap_bounds(dw, 32)
                        ia = _rect(xt8s, gb + 512 + (r0 + dh - 1) * 32 + (c0 + dw - 1),
                                   [[0, 2], [32, nr], [1, nw]])
                        oa = _rect(psum, (r0 - row_lo) * 32 + c0, [[32, nr], [1, nw]])
                        nc.tensor.matmul(oa, dg, ia, start=(i == 0), stop=(i == 8),
                                         perf_mode=PM.DoubleRow,
                                         skip_group_check=True)

            def conv_kv_mms(psum, xsrc, g, fp8):
                gb = g * L
                n_mm = 27 if fp8 else 18
                mm_i = 0
                for dh, dw in TAPS:
                    t = 3 * dh + dw
                    dg = (_slot(dgk_sb[:, g, t, :], (9 - t) * 128) if fp8
                          else dgv_sb[:, g, t, :])
                    slot = [[0, 2]] if fp8 else []
                    pm = PM.DoubleRow if fp8 else None
                    r0, nr = _tap_bounds_s2(dh, 16)
                    c0, nw = _tap_bounds_s2(dw, 16)
                    if fp8:
                        for ib, ob in ((0, 0), (256, 64)):
                            ia = _rect(xsrc, gb + ib + (2 * r0 + dh - 1) * 16
                                       + (2 * c0 + dw - 1),
                                       slot + [[32, nr], [2, nw]])
                            oa = _rect(psum, ob + r0 * 8 + c0, [[8, nr], [1, nw]])
                            nc.tensor.matmul(oa, dg, ia, start=(mm_i == 0),
                                             stop=False, perf_mode=pm,
                                             skip_group_check=True)
                            mm_i += 1
                    else:
                        ia = _rect(xsrc, gb + (2 * r0 + dh - 1) * 16 + (2 * c0 + dw - 1),
                                   [[256, 2], [32, nr], [2, nw]])
                        oa = _rect(psum, r0 * 8 + c0, [[64, 2], [8, nr], [1, nw]])
                        nc.tensor.matmul(oa, dg, ia, start=(mm_i == 0), stop=False,
                                         perf_mode=pm, skip_group_check=True)
                        mm_i += 1
                    r0, nr = _tap_bounds_s2(dh, 32)
                    c0, nw = _tap_bounds_s2(dw, 32)
                    ia = _rect(xsrc, gb + 512 + (2 * r0 + dh - 1) * 32 + (2 * c0 + dw - 1),
                               slot + [[64, nr], [2, nw]])
                    oa = _rect(psum, 128 + r0 * 16 + c0, [[16, nr], [1, nw]])
                    nc.tensor.matmul(oa, dg, ia, start=False, stop=(mm_i == n_mm - 1),
                                     perf_mode=pm, skip_group_check=True)
                    mm_i += 1

            # ------------- per-batch stage emitters -------------
            state = {}

            def emit_dma_stage(b, rb):
                st = {}
                st["xt"] = actp.tile([128, G, L], BF16, name=f"xt_{rb}", tag="xt",
                                     bufs=2)
                st["xt8"] = actp.tile([128, G, L], F8, name=f"xt8_{rb}", tag="xt8",
                                      bufs=2)
                xbf = dramp.tile([L, D], BF16, name=f"xbf_{rb}", tag="xbf", bufs=2)
                for g in range(G):
                    nc.gpsimd.dma_start(out=xbf[:, g * 128:(g + 1) * 128],
                                        in_=x_d[b][:, g * 128:(g + 1) * 128])
                    nc.sync.dma_start_transpose(
                        st["xt"][:, g, :], xbf[:, g * 128:(g + 1) * 128])
                    if rb == "0":
                        # startup: Pool queue is busy with consts; DVE and ACT
                        # are idle -- split the casts across both
                        if g % 2 == 0:
                            nc.vector.tensor_copy(out=st["xt8"][:, g],
                                                  in_=st["xt"][:, g])
                        else:
                            nc.scalar.copy(st["xt8"][:, g], st["xt"][:, g])
                    else:
                        nc.gpsimd.tensor_copy(out=st["xt8"][:, g], in_=st["xt"][:, g])
                st["cq8"] = actp.tile([128, G, L], F8, name=f"cq8_{rb}", tag="cq8",
                                      bufs=2)
                st["ck8"] = actp.tile([128, G, LKV], F8, name=f"ck8_{rb}", tag="ck8",
                                      bufs=2)
                st["cv"] = actp.tile([128, G, LKV], BF16, name=f"cv_{rb}", tag="cv",
                                     bufs=2)
                st["kt8"] = actp.tile([128, G + 1, LKV], F8, name=f"kt8_{rb}",
                                      tag="kt8", bufs=2)
                st["v8"] = actp.tile([128, 3, NH, 2, 128], F8, name=f"v8_{rb}",
                                     tag="v8", bufs=2)
                if int(rb) < 2:
                    # ones/zero blocks live in disjoint columns from the evacs,
                    # so each pool buffer only needs them written once
                    nc.gpsimd.memset(st["kt8"][:, G, :], 0.0)
                    nc.gpsimd.memset(_rect(st["v8"], HD,
                                           [[3072, 3], [256, NH], [1, HD]]), 1.0)
                    nc.gpsimd.memset(_rect(st["v8"], 128 + HD,
                                           [[3072, 3], [256, NH], [1, HD]]), 0.0)
                state[b] = st

            def h1_chunks(b, rb):
                """PE-dense conv + k/v projection work, as filler closures."""
                st = state[b]
                chunks = []
                for g in range(G):
                    def convq_g(g=g):
                        for ct in range(3):
                            pj = ps_pj.tile([128, 512], F32,
                                            name=f"cqp_{rb}_{g}_{ct}", tag="pj")
                            conv_q_mms(pj, st["xt8"], g, ct)
                            nc.vector.tensor_copy(
                                out=st["cq8"][:, g, ct * 512:(ct + 1) * 512], in_=pj)
                    chunks.append(convq_g)
                for g in range(G):
                    def convkv_g(g=g):
                        pk = ps_pj.tile([128, LKV], F32, name=f"ckp_{rb}_{g}",
                                        tag="pj")
                        conv_kv_mms(pk, st["xt8"], g, fp8=True)
                        nc.scalar.copy(st["ck8"][:, g, :], pk)
                        pv = ps_pj.tile([128, LKV], F32, name=f"cvp_{rb}_{g}",
                                        tag="pj")
                        conv_kv_mms(pv, st["xt"], g, fp8=False)
                        nc.scalar.copy(st["cv"][:, g, :], pv)
                    chunks.append(convkv_g)
                for go in range(G):
                    def kproj_go(go=go):
                        pj = ps_pj.tile([128, LKV], F32, name=f"kpj_{rb}_{go}",
                                        tag="pj")
                        for m in range(3):
                            lw = _slot(wk8_sb[:, 2 * m, go * 128:(go + 1) * 128], D)
                            ra = _slot(st["ck8"][:, 2 * m, :], LKV)
                            nc.tensor.matmul(pj, lw, ra, start=(m == 0), stop=(m == 2),
                                             perf_mode=PM.DoubleRow)
                        nc.vector.tensor_copy(out=st["kt8"][:, go, :], in_=pj)
                    chunks.append(kproj_go)
                for mt in range(3):
                    def vproj_mt(mt=mt):
                        for nh in range(2):
                            pj = ps_pj.tile([128, LKV], F32,
                                            name=f"vpj_{rb}_{mt}_{nh}", tag="pj")
                            for g in range(G):
                                nc.tensor.matmul(
                                    pj, st["cv"][:, g, mt * 128:(mt + 1) * 128],
                                    wv_sb[:, g, nh * 384:(nh + 1) * 384],
                                    start=(g == 0), stop=(g == G - 1))
                            hi = _rect(st["v8"], mt * 3072 + nh * 6 * 256,
                                       [[256, 6], [1, HD]])
                            nc.scalar.copy(hi, pj)
                            lo = _rect(st["v8"], mt * 3072 + nh * 6 * 256 + 128,
                                       [[256, 6], [1, HD]])
                            nc.vector.tensor_tensor(out=lo, in0=pj, in1=hi,
                                                    op=OP.subtract)
                    chunks.append(vproj_mt)
                return chunks

            def emit_h2(b, rb, filler, pending):
                """Latency-bound attention; drains filler chunks (next batch's
                conv work) into the PE stream to cover softmax latency."""
                st = state[b]
                cq8, kt8, v8 = st["cq8"], st["kt8"], st["v8"]

                def drain(n=1):
                    for _ in range(n):
                        c = next(filler, None)
                        if c is not None:
                            c()

                def emit_outproj(tp, xatt_t, mt2):
                    onat = roll.tile([128, D], F32, name=f"on_{rb}_{tp}_{mt2}",
                                     tag="onat", bufs=3)
                    for nh in range(2):
                        pj = ps_pj.tile([128, 384], F32,
                                        name=f"opj_{rb}_{tp}_{mt2}_{nh}", tag="pj")
                        nc.tensor.matmul(pj, ones1b_sb,
                                         bfinb_sb[:, nh * 384:(nh + 1) * 384],
                                         start=True, stop=False)
                        for g in range(G):
                            nc.tensor.matmul(
                                pj, xatt_t[:, g, mt2 * 128:(mt2 + 1) * 128],
                                wp_sb[:, g, nh * 384:(nh + 1) * 384],
                                start=False, stop=(g == G - 1))
                        if nh == 0:
                            nc.scalar.copy(onat[:, 0:384], pj)
                        else:
                            nc.vector.tensor_copy(out=onat[:, 384:768], in_=pj)
                    tok0 = tp * 512 + mt2 * 128
                    nc.sync.dma_start(out=out_d[b, tok0:tok0 + 128, :], in_=onat)

                def qproj_chunk(t, qt8, go, pool=None):
                    pj = (pool or ps_pj).tile([128, 512], F32,
                                              name=f"qpj_{rb}_{t}_{go}",
                                              tag="pj" if pool is None else "u")
                    for m in range(3):
                        lw = _slot(wq8_sb[:, 2 * m, go * 128:(go + 1) * 128], D)
                        ra = _slot(cq8[:, 2 * m, t * 512:(t + 1) * 512], L)
                        nc.tensor.matmul(pj, lw, ra, start=(m == 0), stop=(m == 2),
                                         perf_mode=PM.DoubleRow)
                    nc.scalar.activation(qt8[:, go, :], pj, AF.Identity,
                                         bias=bq_sb[:, go:go + 1])

                xatt_tiles = {}
                qt8_next = roll.tile([128, G, 512], F8, name=f"qt8_{rb}_0",
                                     tag="qt8", bufs=2)
                for go in range(G):
                    # ps_u is idle between batches; avoids colliding with
                    # ps_pj buffers still draining the previous conv filler
                    qproj_chunk(0, qt8_next, go, pool=ps_u)
                for t in range(3):
                    qt8 = qt8_next
                    if t < 2:
                        qt8_next = roll.tile([128, G, 512], F8,
                                             name=f"qt8_{rb}_{t + 1}",
                                             tag="qt8", bufs=2)
                    drain(1)

                    xatt = roll.tile([128, G, 512], BF16, name=f"xatt_{rb}_{t}",
                                     tag="xatt", bufs=2)
                    xatt_tiles[t] = xatt
                    kcs = (0,) if t == 0 else (0, 1, 2)
                    for hp in range(6):
                        g = hp
                        ups = [ps_u.tile([128, 512], F32,
                                         name=f"u_{rb}_{t}_{hp}_{j}", tag="u")
                               for j in range(2)]

                        def emit_scores(kc):
                            sps = ps_big.tile([128, 1024], F32,
                                              name=f"s_{rb}_{t}_{hp}_{kc}", tag="big")
                            for j, po in enumerate((0, HD)):
                                zoff = (G - g) * LKV - kc * 128
                                lw = _slot(kt8[po:po + HD, g,
                                               kc * 128:(kc + 1) * 128], zoff)
                                ra = _slot(qt8[po:po + HD, g, :], 0)
                                nc.tensor.matmul(
                                    sps[:, j * 512:(j + 1) * 512], lw, ra,
                                    start=True, stop=True, perf_mode=PM.DoubleRow)
                            aT8 = roll.tile([128, 1024], F8,
                                            name=f"aT_{rb}_{t}_{hp}_{kc}",
                                            tag="aT", bufs=6)
                            nc.scalar.activation(aT8, sps, AF.Exp, scale=SCALE,
                                                 bias=shift_sb)
                            return aT8

                        def emit_av(kc, aT8, i):
                            for j in range(2):
                                lw = v8[:, kc, 2 * hp + j]
                                ra = _slot(aT8[:, j * 512:(j + 1) * 512], 0)
                                nc.tensor.matmul(ups[j], lw, ra,
                                                 start=(i == 0),
                                                 stop=(i == len(kcs) - 1),
                                                 perf_mode=PM.DoubleRow)

                        aTs = []
                        for i, kc in enumerate(kcs):
                            aTs.append(emit_scores(kc))
                            if i >= 1:
                                emit_av(kcs[i - 1], aTs[i - 1], i - 1)
                        emit_av(kcs[-1], aTs[-1], len(kcs) - 1)
                        for j in range(2):
                            recip = roll.tile([HD, 512], F32,
                                              name=f"rc_{rb}_{t}_{hp}_{j}",
                                              tag="recip", bufs=3)
                            nc.vector.reciprocal(recip, ups[j][HD:2 * HD, :])
                            nc.vector.tensor_mul(
                                xatt[j * HD:(j + 1) * HD, g, :],
                                ups[j][0:HD, :], recip)
                        drain(1)
                        if t < 2:
                            qproj_chunk(t + 1, qt8_next, hp)
                        if t > 0 and 1 <= hp <= 4:
                            emit_outproj(t - 1, xatt_tiles[t - 1], hp - 1)
                        elif t == 0 and 1 <= hp <= 4 and pending:
                            pending.pop(0)()
                while pending:
                    pending.pop(0)()
                # defer this batch's last-tile outproj into the next batch's
                # t=0 attention window (PE filler there); emit inline if last.
                xt2 = xatt_tiles[2]
                new_pending = [
                    (lambda mt2=mt2: emit_outproj(2, xt2, mt2)) for mt2 in range(4)]
                return new_pending

            # ------------- pipelined schedule -------------
            bseq = [bb for _ in range(reps) for bb in range(nb)]
            n = len(bseq)
            emit_dma_stage(bseq[0], "0")
            emit_const_dmas()
            for c in h1_chunks(bseq[0], "0"):
                c()
            pending = []
            for i in range(n):
                if i + 1 < n:
                    emit_dma_stage(bseq[i + 1], f"{i + 1}")
                    filler = iter(h1_chunks(bseq[i + 1], f"{i + 1}"))
                else:
                    filler = iter(())
                pending = emit_h2(bseq[i], f"{i}", filler, pending)
                for c in filler:
                    c()
                state.pop(bseq[i], None)
            for c in pending:
                c()

    nc.compile()
    return nc


_NC_CACHE = {}


def _get_program():
    if "nc" not in _NC_CACHE:
        _NC_CACHE["nc"] = build_program()
    return _NC_CACHE["nc"]


def _host_prep(inputs):
    import ml_dtypes
    MLF8 = ml_dtypes.float8_e4m3
    MLBF = ml_dtypes.bfloat16

    f = lambda k: np.asarray(inputs[k], dtype=np.float32)
    w = {}
    effs = {}
    for n in ("q", "k", "v"):
        inv = f(f"bn_{n}_g") / np.sqrt(f(f"bn_{n}_v") + EPS)
        beta = f(f"bn_{n}_b") - f(f"bn_{n}_m") * inv
        effs[n] = (inv, beta)
    wq, wk, wv, wp = f("wq"), f("wk"), f("wv"), f("w_proj")
    wqT = np.ascontiguousarray((wq * effs["q"][0][None, :]).T)  # [in, out]
    wkT = np.ascontiguousarray((wk * effs["k"][0][None, :]).T)
    wvT = np.ascontiguousarray((wv * effs["v"][0][None, :]).T)
    wpT = np.ascontiguousarray(wp.T)
    w["wq8"] = wqT.reshape(G, 128, D).transpose(1, 0, 2).astype(MLF8)
    w["wk8"] = wkT.reshape(G, 128, D).transpose(1, 0, 2).astype(MLF8)
    w["wv"] = wvT.reshape(G, 128, D).transpose(1, 0, 2).astype(MLBF)
    w["wp"] = wpT.reshape(G, 128, D).transpose(1, 0, 2).astype(MLBF)
    bq_eff = wq @ effs["q"][1]
    w["bq"] = np.ascontiguousarray(bq_eff.reshape(G, 128).T.astype(np.float32))
    bv_eff = wv @ effs["v"][1]
    b_fin = (f("b_proj") + wp @ bv_eff).astype(np.float32)
    bf_hi = b_fin.astype(MLF8)
    bf_lo = (b_fin - bf_hi.astype(np.float32)).astype(MLF8)
    bfin8 = np.zeros((1, 2, D), MLF8)
    bfin8[0, 0] = bf_hi
    bfin8[0, 1] = bf_lo
    w["bfin8"] = bfin8
    w["bfinb"] = b_fin.reshape(1, D).astype(MLBF)
    w["ones18"] = np.ones((1, 2, 128), np.float32).astype(MLF8)
    eye = np.eye(128, dtype=np.float32)
    for name, key in (("dgq", "conv_q_w"), ("dgk", "conv_k_w")):
        cw = f(key).reshape(D, 9)
        dg = np.zeros((128, G, 10, 128), np.float32)
        for g in range(G):
            dg[:, g, :9, :] = eye[:, None, :] * cw[g * 128:(g + 1) * 128][:, :, None]
        w[name] = dg.astype(MLF8)
    cwv = f("conv_v_w").reshape(D, 9)
    dgv = np.zeros((128, G, 9, 128), np.float32)
    for g in range(G):
        dgv[:, g, :, :] = eye[:, None, :] * cwv[g * 128:(g + 1) * 128][:, :, None]
    w["dgv"] = dgv.astype(MLBF)
    return {k: np.ascontiguousarray(v) for k, v in w.items()}


def kernel(**inputs):
    x = np.asarray(inputs["x"], dtype=np.float32)
    assert x.shape == (32, L, D), x.shape
    const = _host_prep(inputs)
    nc = _get_program()
    in_maps = []
    for c in range(8):
        m = dict(const)
        m["x"] = np.ascontiguousarray(x[c * NB:(c + 1) * NB])
        in_maps.append(m)
    res = run_bass_kernel_spmd(nc, in_maps, list(range(8)))
    out = np.concatenate([res.results[c]["out"] for c in range(8)], axis=0)
    return out.astype(np.float32)
